# revision 7
# baseline (speedup 1.0000x reference)
"""Additive attention (nn_AdditiveAttention) on 8 Trainium2 NeuronCores.

Math (per batch b):
  qp = queries[b] @ W_q ; kp = keys[b] @ W_k        # (L, H)
  S[q,k] = sum_h w_v[h] * tanh(qp[q,h] + kp[k,h])
  out[b] = softmax_k(S, masked to k < valid_lens[b]) @ values[b]

Approximation: tanh(x) ~= c0*x + sum_{m=1..3} a_m sin(m w x).
  - The linear term's q-part cancels in softmax; its k-part
    v_k = c0 * (keys @ W_k @ w_v) is host-precomputed and folded into
    the exp bias (scores land PSUM-[k,q], so a per-partition bias).
  - sin(mw(q+k)) = sin_m(q)cos_m(k) + cos_m(q)sin_m(k): one PE matmul
    contraction per product, depth 2*3*H over separable factors.
  - seeds on ScalarE: s1 = Sin(w*p), hh = Sin(w/2*p); cos1 = 1-2*hh^2
    (exact half-angle -- keeps every Sin argument inside the HW window).
  - m=2,3 by product identities on DVE: 2cos2 = 2-4*s1^2,
    sin2/2 = s1*c1, cos3 = (2cos2-1)*c1, sin3 = (2cos2+1)*s1 --
    fused scalar_tensor_tensor / two-op tensor_scalar instructions.
  - w_v*a_m folded into k-halves in place (per-hb tensor_scalar with a
    [128,1] column); m>=2 folds ride on GpSimd to unload DVE.
  - scores: 12-matmul PSUM accumulation group per (slot, kb);
    exp on ScalarE with bias = v-column; masked [values|1] matmul,
    DVE reciprocal + scale, fp16 output, one DMA per slot.

SPMD: one NEFF on 8 cores, 2 slots (one batch per core per slot).
Host sorts batches by valid_len; slot K = max valid_len of the slot.
"""

import sys

if "/opt/trn_rl_repo" not in sys.path:
    sys.path.insert(0, "/opt/trn_rl_repo")

import numpy as np

import concourse.bacc as bacc
import concourse.mybir as mybir
import concourse.tile as tile
from concourse.bass_utils import run_bass_kernel_spmd

N_CORES = 8
B, LQ, LK = 16, 256, 256
D = 256
H = 256
DV = 256
F32 = mybir.dt.float32
F16 = mybir.dt.float16

OMEGA = 0.8726646259971648          # 2*pi/7.2
C0 = 0.2757509648799896
A1 = 0.47873538732528687
A2 = 0.10933910310268402
A3 = 0.033587608486413956

Alu = mybir.AluOpType
ActF = mybir.ActivationFunctionType


def _plan(valid_lens):
    pieces = sorted(range(B), key=lambda b: -int(valid_lens[b]))
    slots = []
    for s in range(B // N_CORES):
        grp = pieces[s * N_CORES:(s + 1) * N_CORES]
        K = max(int(valid_lens[b]) for b in grp)
        K = min(LK, (K + 3) // 4 * 4)
        slots.append((K, grp))
    return slots


def _build(slot_ks):
    K0, K1 = slot_ks
    KBs = [(K + 127) // 128 for K in slot_ks]
    nc = bacc.Bacc("TRN2", target_bir_lowering=False, debug=False,
                   num_devices=N_CORES)
    # arenaA: [wqk(1024) | qT_d0(256) | qT_d1(256) | kT_d0(K0) | kT_d1(K0)]
    AW = 1024 + 2 * LQ + 2 * K0
    arA_ext = nc.dram_tensor("arA", [128, AW], F16, kind="ExternalInput").ap()
    # arenaB: [qkT1(512+2K1) | vx0(KB0*257) | vx1(KB1*257)]
    BW = 2 * LQ + 2 * K1 + KBs[0] * (DV + 1) + KBs[1] * (DV + 1)
    arB_ext = nc.dram_tensor("arB", [128, BW], F16, kind="ExternalInput").ap()
    wvs_ext = nc.dram_tensor("wvs", [128, 6], F32, kind="ExternalInput").ap()
    vcol_ext = nc.dram_tensor("vcol", [128, 4], F32,
                              kind="ExternalInput").ap()
    out_exts = [nc.dram_tensor(f"out{su}", [128, 2 * DV], F16,
                               kind="ExternalOutput").ap()
                for su in range(2)]

    with tile.TileContext(nc) as tc:
        with (
            tc.tile_pool(name="consts", bufs=1) as consts,
            tc.tile_pool(name="sb", bufs=1) as sb,
            tc.tile_pool(name="post", bufs=2) as postp,
            tc.tile_pool(name="pjp", bufs=2, space="PSUM") as pjp,
            tc.tile_pool(name="scp", bufs=2, space="PSUM") as scpp,
            tc.tile_pool(name="avp", bufs=2, space="PSUM") as avp,
        ):
            arA = consts.tile([128, AW], F16, tag="arA", name="arA")
            arB = consts.tile([128, BW], F16, tag="arB", name="arB")
            wvs = consts.tile([128, 6], F32, tag="wvs", name="wvs")
            vcol = consts.tile([128, 4], F32, tag="vcol", name="vcol")

            # input DMAs: 4 queues; critical (wqk+qkT0) split across two
            nc.sync.dma_start(arA[:, 0:1024], arA_ext[:, 0:1024])
            nc.gpsimd.dma_start(arA[:, 1024:], arA_ext[:, 1024:])
            nc.scalar.dma_start(arB[:], arB_ext)
            nc.gpsimd.dma_start(wvs[:], wvs_ext)
            nc.gpsimd.dma_start(vcol[:], vcol_ext)

            def wq(d, hb):
                return arA[:, d * 256 + hb * 128:d * 256 + hb * 128 + 128]

            def wk(d, hb):
                return arA[:, 512 + d * 256 + hb * 128:
                           512 + d * 256 + hb * 128 + 128]

            def qkT(su, part, d):
                # part 0 = q (LQ cols), part 1 = k (K cols)
                K = slot_ks[su]
                base = 1024 if su == 0 else 0
                src = arA if su == 0 else arB
                off = base + (d * LQ if part == 0 else 2 * LQ + d * K)
                n = LQ if part == 0 else K
                return src[:, off:off + n]

            vx_base = 2 * LQ + 2 * K1

            def vx(su, kb):
                off = vx_base + (KBs[0] * (DV + 1) if su else 0) \
                    + kb * (DV + 1)
                return arB[:, off:off + DV + 1]

            # fused trig tiles per slot: [q_hb0|q_hb1|k_hb0|k_hb1]
            KO = 2 * LQ
            F = {}
            for su, K in enumerate(slot_ks):
                W = 2 * LQ + 2 * K
                for nm in ("F1s", "Fh", "F1c", "C4", "F3c", "F3s"):
                    F[su, nm] = sb.tile([128, W], F16, tag=f"{nm}{su}",
                                        name=f"{nm}{su}")
                F[su, "S2q"] = sb.tile([128, 2 * LQ], F16, tag=f"S2q{su}",
                                       name=f"S2q{su}")
                F[su, "K2s"] = sb.tile([128, 2 * K], F16, tag=f"K2s{su}",
                                       name=f"K2s{su}")
                F[su, "expT"] = sb.tile([128, 2 * LQ], F16, tag=f"e{su}",
                                        name=f"e{su}")
                F[su, "ot"] = sb.tile([128, 2 * DV], F16, tag=f"ot{su}",
                                      name=f"ot{su}")

            pskq = {}

            def proj_block(su):
                K = slot_ks[su]
                p = pjp.tile([128, 1024], F32, tag="pj", name=f"pj{su}")
                pskq[su] = p
                for hb in range(2):
                    for d in range(2):
                        nc.tensor.matmul(p[:, hb * 256:hb * 256 + 256],
                                         wq(d, hb), qkT(su, 0, d),
                                         start=(hb == 0 and d == 0),
                                         stop=False, skip_group_check=True)
                for hb in range(2):
                    for d in range(2):
                        nc.tensor.matmul(p[:, 512 + hb * K:512 + hb * K + K],
                                         wk(d, hb), qkT(su, 1, d),
                                         start=(hb == 0 and d == 0),
                                         stop=(hb == 1 and d == 1),
                                         skip_group_check=True)

            def sins_block(su):
                K = slot_ks[su]
                W = 2 * LQ + 2 * K
                p = pskq[su]
                nc.scalar.activation(F[su, "F1s"][:], p[:, 0:W], ActF.Sin,
                                     bias=0.0, scale=OMEGA)
                nc.scalar.activation(F[su, "Fh"][:], p[:, 0:W], ActF.Sin,
                                     bias=0.0, scale=OMEGA / 2.0)

            def dve_block(su):
                K = slot_ks[su]
                f1s, fh, f1c = F[su, "F1s"], F[su, "Fh"], F[su, "F1c"]
                c4, f3c, f3s = F[su, "C4"], F[su, "F3c"], F[su, "F3s"]
                nc.vector.tensor_tensor(f1c[:], fh[:], fh[:], Alu.mult)
                nc.vector.tensor_scalar(f1c[:], f1c[:], -2.0, 1.0,
                                        Alu.mult, Alu.add)
                nc.vector.tensor_tensor(c4[:], f1s[:], f1s[:], Alu.mult)
                nc.vector.tensor_scalar(c4[:], c4[:], -4.0, 2.0,
                                        Alu.mult, Alu.add)
                nc.gpsimd.tensor_tensor(F[su, "S2q"][:], f1s[:, 0:2 * LQ],
                                        f1c[:, 0:2 * LQ], Alu.mult)
                nc.vector.scalar_tensor_tensor(f3c[:], c4[:], -1.0, f1c[:],
                                               Alu.add, Alu.mult)
                nc.vector.scalar_tensor_tensor(f3s[:], c4[:], 1.0, f1s[:],
                                               Alu.add, Alu.mult)
                for hb in range(2):
                    ks = slice(KO + hb * K, KO + hb * K + K)
                    # K2s = (s1k * a2wv) * c1k  (before in-place folds)
                    nc.vector.scalar_tensor_tensor(
                        F[su, "K2s"][:, hb * K:hb * K + K],
                        f1s[:, ks], wvs[:, 2 + hb:3 + hb], f1c[:, ks],
                        Alu.mult, Alu.mult)
                for hb in range(2):
                    ks = slice(KO + hb * K, KO + hb * K + K)
                    nc.vector.tensor_scalar_mul(f1s[:, ks], f1s[:, ks],
                                                wvs[:, hb:hb + 1])
                    nc.vector.tensor_scalar_mul(f1c[:, ks], f1c[:, ks],
                                                wvs[:, hb:hb + 1])
                    nc.vector.tensor_scalar_mul(c4[:, ks], c4[:, ks],
                                                wvs[:, 2 + hb:3 + hb])
                    nc.vector.tensor_scalar_mul(f3c[:, ks], f3c[:, ks],
                                                wvs[:, 4 + hb:5 + hb])
                    nc.vector.tensor_scalar_mul(f3s[:, ks], f3s[:, ks],
                                                wvs[:, 4 + hb:5 + hb])

            scps = {}

            def scores_block(su):
                K = slot_ks[su]
                KB = KBs[su]
                scp = scpp.tile([128, 2 * LQ], F32, tag="sc", name=f"sc{su}")
                scps[su] = scp
                pairs = (("F1c", "F1s"), ("F1s", "F1c"),
                         ("C4", "S2q"), ("K2s", "C4"),
                         ("F3c", "F3s"), ("F3s", "F3c"))
                for kb in range(KB):
                    kr = min(128, K - kb * 128)
                    n = 0
                    for stat_nm, mov_nm in pairs:
                        for hb in range(2):
                            st = F[su, stat_nm]
                            if stat_nm == "K2s":
                                stat = st[:, hb * K + kb * 128:
                                          hb * K + kb * 128 + kr]
                            else:
                                stat = st[:, KO + hb * K + kb * 128:
                                          KO + hb * K + kb * 128 + kr]
                            mov = F[su, mov_nm][:, hb * 256:hb * 256 + 256]
                            nc.tensor.matmul(
                                scp[:kr, kb * 256:kb * 256 + 256],
                                stat, mov, start=(n == 0), stop=(n == 11),
                                skip_group_check=True)
                            n += 1

            def exp_block(su):
                K = slot_ks[su]
                KB = KBs[su]
                scp = scps[su]
                for kb in range(KB):
                    kr = min(128, K - kb * 128)
                    nc.scalar.activation(
                        F[su, "expT"][:kr, kb * 256:kb * 256 + 256],
                        scp[:kr, kb * 256:kb * 256 + 256], ActF.Exp,
                        bias=vcol[:kr, su * 2 + kb:su * 2 + kb + 1],
                        scale=1.0)

            def av_block(su, dma_eng):
                K = slot_ks[su]
                KB = KBs[su]
                expT = F[su, "expT"]
                ot = F[su, "ot"]
                for qb in range(2):
                    av = avp.tile([128, 512], F32, tag="av", name="av")
                    for kb in range(KB):
                        kr = min(128, K - kb * 128)
                        nc.tensor.matmul(
                            av[:, 0:DV + 1],
                            expT[:kr, kb * 256 + qb * 128:
                                 kb * 256 + qb * 128 + 128],
                            vx(su, kb)[:kr, :],
                            start=(kb == 0), stop=(kb == KB - 1))
                    rec = postp.tile([128, 1], F32, tag="rec", name="rec")
                    nc.vector.reciprocal(rec[:], av[:, DV:DV + 1])
                    if qb == 0:
                        nc.vector.tensor_scalar_mul(
                            ot[:, qb * DV:qb * DV + DV],
                            av[:, 0:DV], rec[:, 0:1])
                    else:
                        nc.scalar.activation(ot[:, qb * DV:qb * DV + DV],
                                             av[:, 0:DV], ActF.Copy,
                                             scale=rec[:, 0:1])
                dma_eng.dma_start(out_exts[su], ot[:])

            proj_block(0)
            sins_block(0)
            proj_block(1)
            sins_block(1)
            dve_block(0)
            scores_block(0)
            exp_block(0)
            dve_block(1)
            scores_block(1)
            av_block(0, nc.sync)
            exp_block(1)
            av_block(1, nc.scalar)
    nc.compile()
    return nc


_CACHE = {}


def _get_graph(slot_ks):
    key = tuple(slot_ks)
    if key not in _CACHE:
        _CACHE[key] = _build(slot_ks)
    return _CACHE[key]


def _build_in_maps(queries, keys, values, valid_lens, W_q, W_k, w_v, slots):
    K0, K1 = slots[0][0], slots[1][0]
    KBs = [(K + 127) // 128 for K in (K0, K1)]
    wqk = np.empty((128, 1024), np.float16)
    for d in range(2):
        for hb in range(2):
            wqk[:, d * 256 + hb * 128:d * 256 + hb * 128 + 128] = \
                W_q[d * 128:(d + 1) * 128,
                    hb * 128:(hb + 1) * 128].astype(np.float16)
            wqk[:, 512 + d * 256 + hb * 128:512 + d * 256 + hb * 128 + 128] \
                = W_k[d * 128:(d + 1) * 128,
                      hb * 128:(hb + 1) * 128].astype(np.float16)
    wvs = np.empty((128, 6), np.float32)
    for m, a in enumerate((A1, A2, A3)):
        for hb in range(2):
            wvs[:, 2 * m + hb] = a * w_v[hb * 128:(hb + 1) * 128]
    wkv = W_k.astype(np.float32) @ (C0 * w_v.astype(np.float32))

    AW = 1024 + 2 * LQ + 2 * K0
    BW = 2 * LQ + 2 * K1 + KBs[0] * (DV + 1) + KBs[1] * (DV + 1)
    in_maps = []
    for c in range(N_CORES):
        arA = np.zeros((128, AW), np.float16)
        arA[:, 0:1024] = wqk
        arB = np.zeros((128, BW), np.float16)
        vcol = np.zeros((128, 4), np.float32)
        for su, (K, grp) in enumerate(slots):
            b = grp[c]
            vl = int(valid_lens[b])
            qT = queries[b].T.astype(np.float16)          # [D, LQ]
            kT = keys[b, :K, :].T.astype(np.float16)      # [D, K]
            base = 1024 if su == 0 else 0
            dst = arA if su == 0 else arB
            for d in range(2):
                dst[:, base + d * LQ:base + (d + 1) * LQ] = \
                    qT[d * 128:(d + 1) * 128]
                dst[:, base + 2 * LQ + d * K:base + 2 * LQ + (d + 1) * K] = \
                    kT[d * 128:(d + 1) * 128]
            KB = KBs[su]
            vpad = np.zeros((KB * 128, DV + 1), np.float16)
            vpad[:vl, :DV] = values[b, :vl, :].astype(np.float16)
            vpad[:vl, DV] = 1.0
            off = 2 * LQ + 2 * K1 + (KBs[0] * (DV + 1) if su else 0)
            arB[:, off:off + KB * (DV + 1)] = np.ascontiguousarray(
                vpad.reshape(KB, 128, DV + 1).transpose(1, 0, 2)
                .reshape(128, KB * (DV + 1)))
            v = (keys[b, :K, :].astype(np.float32) @ wkv)  # [K]
            for kb in range(KB):
                kr = min(128, K - kb * 128)
                vcol[:kr, su * 2 + kb] = v[kb * 128:kb * 128 + kr]
        in_maps.append({"arA": arA, "arB": arB, "wvs": wvs, "vcol": vcol})
    return in_maps


def kernel(queries, keys, values, valid_lens, W_q, W_k, w_v):
    queries = np.asarray(queries, dtype=np.float32)
    keys = np.asarray(keys, dtype=np.float32)
    values = np.asarray(values, dtype=np.float32)
    valid_lens = np.asarray(valid_lens)
    W_q = np.asarray(W_q, dtype=np.float32)
    W_k = np.asarray(W_k, dtype=np.float32)
    w_v = np.asarray(w_v, dtype=np.float32)

    slots = _plan(valid_lens)
    nc = _get_graph([K for (K, _) in slots])
    in_maps = _build_in_maps(queries, keys, values, valid_lens,
                             W_q, W_k, w_v, slots)
    res = run_bass_kernel_spmd(nc, in_maps, list(range(N_CORES)))

    out = np.empty((B, LQ, DV), np.float32)
    for su, (K, grp) in enumerate(slots):
        for c, b in enumerate(grp):
            o = res.results[c][f"out{su}"]        # [128, 2*DV] fp16
            out[b] = (o.reshape(128, 2, DV).transpose(1, 0, 2)
                      .reshape(LQ, DV).astype(np.float32))
    return out


# revision 13
# speedup vs baseline: 1.0396x; 1.0396x over previous
"""Additive attention (nn_AdditiveAttention) on 8 Trainium2 NeuronCores.

Math (per batch b):
  qp = queries[b] @ W_q ; kp = keys[b] @ W_k        # (L, H)
  S[q,k] = sum_h w_v[h] * tanh(qp[q,h] + kp[k,h])
  out[b] = softmax_k(S, masked to k < valid_lens[b]) @ values[b]

Approximation: tanh(x) ~= c0*x + sum_{m=1..3} a_m sin(m w x).
  - The linear term's q-part cancels in softmax; its k-part
    v_k = c0 * (keys @ W_k @ w_v) is host-precomputed and folded into
    the exp bias (scores land PSUM-[k,q], so a per-partition bias).
  - sin(mw(q+k)) = sin_m(q)cos_m(k) + cos_m(q)sin_m(k): one PE matmul
    contraction per product, depth 2*3*H over separable factors.
  - seeds on ScalarE: s1 = Sin(w*p), hh = Sin(w/2*p); cos1 = 1-2*hh^2
    (exact half-angle -- keeps every Sin argument inside the HW window).
  - m=2,3 by product identities on DVE: 2cos2 = 2-4*s1^2,
    sin2/2 = s1*c1, cos3 = (2cos2-1)*c1, sin3 = (2cos2+1)*s1 --
    fused scalar_tensor_tensor / two-op tensor_scalar instructions.
  - w_v*a_m folded into k-halves in place (per-hb tensor_scalar with a
    [128,1] column); m>=2 folds ride on GpSimd to unload DVE.
  - scores: 12-matmul PSUM accumulation group per (slot, kb);
    exp on ScalarE with bias = v-column; masked [values|1] matmul,
    DVE reciprocal + scale, fp16 output, one DMA per slot.

SPMD: one NEFF on 8 cores, 2 slots (one batch per core per slot).
Host sorts batches by valid_len; slot K = max valid_len of the slot.
"""

import sys

if "/opt/trn_rl_repo" not in sys.path:
    sys.path.insert(0, "/opt/trn_rl_repo")

import numpy as np

import concourse.bacc as bacc
import concourse.mybir as mybir
import concourse.tile as tile
from concourse.bass_utils import run_bass_kernel_spmd

N_CORES = 8
B, LQ, LK = 16, 256, 256
D = 256
H = 256
DV = 256
F32 = mybir.dt.float32
F16 = mybir.dt.float16

OMEGA = 0.8726646259971648          # 2*pi/7.2
C0 = 0.2757509648799896
A1 = 0.47873538732528687
A2 = 0.10933910310268402
A3 = 0.033587608486413956

Alu = mybir.AluOpType
ActF = mybir.ActivationFunctionType


def _plan(valid_lens):
    pieces = sorted(range(B), key=lambda b: -int(valid_lens[b]))
    slots = []
    for s in range(B // N_CORES):
        grp = pieces[s * N_CORES:(s + 1) * N_CORES]
        K = max(int(valid_lens[b]) for b in grp)
        K = min(LK, (K + 3) // 4 * 4)
        slots.append((K, grp))
    return slots


def _build(slot_ks):
    K0, K1 = slot_ks
    KBs = [(K + 127) // 128 for K in slot_ks]
    nc = bacc.Bacc("TRN2", target_bir_lowering=False, debug=False,
                   num_devices=N_CORES)
    # arenaA: [wqk(1024) | qT_d0(256) | qT_d1(256) | kT_d0(K0) | kT_d1(K0)]
    AW = 1024 + 2 * LQ + 2 * K0
    arA_ext = nc.dram_tensor("arA", [128, AW], F16, kind="ExternalInput").ap()
    # arenaB: [qkT1(512+2K1) | vx0(KB0*257) | vx1(KB1*257)]
    BW = 2 * LQ + 2 * K1 + KBs[0] * (DV + 1) + KBs[1] * (DV + 1)
    arB_ext = nc.dram_tensor("arB", [128, BW], F16, kind="ExternalInput").ap()
    wvs_ext = nc.dram_tensor("wvs", [128, 6], F32, kind="ExternalInput").ap()
    vcol_ext = nc.dram_tensor("vcol", [128, 4], F32,
                              kind="ExternalInput").ap()
    out_exts = [nc.dram_tensor(f"out{su}", [128, 2 * DV], F16,
                               kind="ExternalOutput").ap()
                for su in range(2)]

    with tile.TileContext(nc) as tc:
        with (
            tc.tile_pool(name="consts", bufs=1) as consts,
            tc.tile_pool(name="sb", bufs=1) as sb,
            tc.tile_pool(name="post", bufs=2) as postp,
            tc.tile_pool(name="pjp", bufs=2, space="PSUM") as pjp,
            tc.tile_pool(name="scp", bufs=2, space="PSUM") as scpp,
            tc.tile_pool(name="avp", bufs=2, space="PSUM") as avp,
        ):
            arA = consts.tile([128, AW], F16, tag="arA", name="arA")
            arB = consts.tile([128, BW], F16, tag="arB", name="arB")
            wvs = consts.tile([128, 6], F32, tag="wvs", name="wvs")
            vcol = consts.tile([128, 4], F32, tag="vcol", name="vcol")

            # critical input (wqk|qT|kT) split as FIRST transfer on each of
            # the 3 DMA queues; slot1/values arena + small consts second
            nc.sync.dma_start(arA[:, 0:1024], arA_ext[:, 0:1024])
            nc.gpsimd.dma_start(arA[:, 1024:1024 + 2 * LQ],
                                arA_ext[:, 1024:1024 + 2 * LQ])
            nc.scalar.dma_start(arA[:, 1024 + 2 * LQ:],
                                arA_ext[:, 1024 + 2 * LQ:])
            nc.scalar.dma_start(arB[:], arB_ext)
            nc.gpsimd.dma_start(wvs[:], wvs_ext)
            nc.gpsimd.dma_start(vcol[:], vcol_ext)

            def wq(d, hb):
                return arA[:, d * 256 + hb * 128:d * 256 + hb * 128 + 128]

            def wk(d, hb):
                return arA[:, 512 + d * 256 + hb * 128:
                           512 + d * 256 + hb * 128 + 128]

            def qkT(su, part, d):
                # part 0 = q (LQ cols), part 1 = k (K cols)
                K = slot_ks[su]
                base = 1024 if su == 0 else 0
                src = arA if su == 0 else arB
                off = base + (d * LQ if part == 0 else 2 * LQ + d * K)
                n = LQ if part == 0 else K
                return src[:, off:off + n]

            vx_base = 2 * LQ + 2 * K1

            def vx(su, kb):
                off = vx_base + (KBs[0] * (DV + 1) if su else 0) \
                    + kb * (DV + 1)
                return arB[:, off:off + DV + 1]

            # fused trig tiles per slot: [q_hb0|q_hb1|k_hb0|k_hb1]
            KO = 2 * LQ
            F = {}
            for su, K in enumerate(slot_ks):
                W = 2 * LQ + 2 * K
                for nm in ("F1s", "Fh", "F1c", "C4", "F3c", "F3s"):
                    F[su, nm] = sb.tile([128, W], F16, tag=f"{nm}{su}",
                                        name=f"{nm}{su}")
                F[su, "S2q"] = sb.tile([128, 2 * LQ], F16, tag=f"S2q{su}",
                                       name=f"S2q{su}")
                F[su, "K2s"] = sb.tile([128, 2 * K], F16, tag=f"K2s{su}",
                                       name=f"K2s{su}")
                F[su, "K2c"] = sb.tile([128, 2 * K], F16, tag=f"K2c{su}",
                                       name=f"K2c{su}")
                F[su, "expT"] = sb.tile([128, 2 * LQ], F16, tag=f"e{su}",
                                        name=f"e{su}")
                F[su, "ot"] = sb.tile([128, 2 * DV], F16, tag=f"ot{su}",
                                      name=f"ot{su}")

            # HAM warmup: dependency-free matmuls fill the DMA-wait window
            # and flip the PE clock gate to 8/8 before real work arrives.
            warm_sb = consts.tile([128, 2 * LQ], F16, tag="warm", name="warm")
            nc.vector.memset(warm_sb[:], 0.125)
            warm_ps = scpp.tile([128, 2 * LQ], F32, tag="sc", name="warmps")
            NWARM = 28
            for i in range(NWARM):
                nc.tensor.matmul(warm_ps[:, 0:256],
                                 warm_sb[:, (i % 2) * 128:(i % 2) * 128 + 128],
                                 warm_sb[:, 256:512],
                                 start=(i == 0), stop=(i == NWARM - 1),
                                 skip_group_check=True)

            pskq = {}

            def proj_block(su):
                K = slot_ks[su]
                p = pjp.tile([128, 1024], F32, tag="pj", name=f"pj{su}")
                pskq[su] = p
                for hb in range(2):
                    for d in range(2):
                        nc.tensor.matmul(p[:, hb * 256:hb * 256 + 256],
                                         wq(d, hb), qkT(su, 0, d),
                                         start=(hb == 0 and d == 0),
                                         stop=False, skip_group_check=True)
                for hb in range(2):
                    for d in range(2):
                        nc.tensor.matmul(p[:, 512 + hb * K:512 + hb * K + K],
                                         wk(d, hb), qkT(su, 1, d),
                                         start=(hb == 0 and d == 0),
                                         stop=(hb == 1 and d == 1),
                                         skip_group_check=True)

            def sins_block(su):
                K = slot_ks[su]
                W = 2 * LQ + 2 * K
                p = pskq[su]
                nc.scalar.activation(F[su, "Fh"][:], p[:, 0:W], ActF.Sin,
                                     bias=0.0, scale=OMEGA / 2.0)
                nc.scalar.activation(F[su, "F1s"][:], p[:, 0:W], ActF.Sin,
                                     bias=0.0, scale=OMEGA)

            def dve_block(su):
                K = slot_ks[su]
                f1s, fh, f1c = F[su, "F1s"], F[su, "Fh"], F[su, "F1c"]
                c4, f3c, f3s = F[su, "C4"], F[su, "F3c"], F[su, "F3s"]
                k2s, k2c = F[su, "K2s"], F[su, "K2c"]
                kk = slice(KO, KO + 2 * K)
                qq = slice(0, 2 * LQ)
                tt = nc.vector.tensor_tensor
                ts = nc.vector.tensor_scalar
                tsm = nc.vector.tensor_scalar_mul
                # slot1's big products ride on GpSimd, overlapping slot0 DVE
                ttuv = nc.gpsimd.tensor_tensor if su == 1 else tt
                # U (into f1c) and V (into c4) read unfolded seeds
                ttuv(f1c[:], fh[:], fh[:], Alu.mult)
                ttuv(c4[:], f1s[:], f1s[:], Alu.mult)
                ts(f1c[:], f1c[:], -2.0, 1.0, Alu.mult, Alu.add)   # cos1
                for hb in range(2):                      # K1s = a1wv*s1k
                    ks = slice(KO + hb * K, KO + hb * K + K)
                    tsm(f1s[:, ks], f1s[:, ks], wvs[:, hb:hb + 1])
                # K2s = (a1wv*s1k)*c1k * (a2/a1)   (c1k still unfolded)
                tt(k2s[:], f1s[:, kk], f1c[:, kk], Alu.mult)
                tsm(k2s[:], k2s[:], A2 / A1)
                for hb in range(2):                      # K1c = a1wv*c1k
                    ks = slice(KO + hb * K, KO + hb * K + K)
                    tsm(f1c[:, ks], f1c[:, ks], wvs[:, hb:hb + 1])
                ts(c4[:], c4[:], -4.0, 2.0, Alu.mult, Alu.add)     # 2cos2
                for hb in range(2):                      # K2c = a2wv*C4k
                    ks = slice(KO + hb * K, KO + hb * K + K)
                    tsm(k2c[:, hb * K:hb * K + K], c4[:, ks],
                        wvs[:, 2 + hb:3 + hb])
                # k-halves of m=3 first (they gate the score stationaries)
                ts(f3c[:, kk], c4[:, kk], 1.0, -1.0, Alu.mult, Alu.add)
                tt(f3c[:, kk], f3c[:, kk], f1c[:, kk], Alu.mult)
                tsm(f3c[:, kk], f3c[:, kk], A3 / A1)     # K3c
                ts(f3s[:, kk], c4[:, kk], 1.0, 1.0, Alu.mult, Alu.add)
                tt(f3s[:, kk], f3s[:, kk], f1s[:, kk], Alu.mult)
                tsm(f3s[:, kk], f3s[:, kk], A3 / A1)     # K3s
                tt(F[su, "S2q"][:], f1s[:, qq], f1c[:, qq], Alu.mult)
                ts(f3c[:, qq], c4[:, qq], 1.0, -1.0, Alu.mult, Alu.add)
                tt(f3c[:, qq], f3c[:, qq], f1c[:, qq], Alu.mult)
                ts(f3s[:, qq], c4[:, qq], 1.0, 1.0, Alu.mult, Alu.add)
                tt(f3s[:, qq], f3s[:, qq], f1s[:, qq], Alu.mult)

            scps = {}

            def scores_block(su):
                K = slot_ks[su]
                KB = KBs[su]
                scp = scpp.tile([128, 2 * LQ], F32, tag="sc", name=f"sc{su}")
                scps[su] = scp
                pairs = (("F1c", "F1s"), ("F1s", "F1c"),
                         ("K2c", "S2q"), ("K2s", "C4"),
                         ("F3c", "F3s"), ("F3s", "F3c"))
                for kb in range(KB):
                    kr = min(128, K - kb * 128)
                    n = 0
                    for stat_nm, mov_nm in pairs:
                        for hb in range(2):
                            st = F[su, stat_nm]
                            if stat_nm in ("K2s", "K2c"):
                                stat = st[:, hb * K + kb * 128:
                                          hb * K + kb * 128 + kr]
                            else:
                                stat = st[:, KO + hb * K + kb * 128:
                                          KO + hb * K + kb * 128 + kr]
                            mov = F[su, mov_nm][:, hb * 256:hb * 256 + 256]
                            nc.tensor.matmul(
                                scp[:kr, kb * 256:kb * 256 + 256],
                                stat, mov, start=(n == 0), stop=(n == 11),
                                skip_group_check=True)
                            n += 1

            def exp_block(su):
                K = slot_ks[su]
                KB = KBs[su]
                scp = scps[su]
                for kb in range(KB):
                    kr = min(128, K - kb * 128)
                    nc.scalar.activation(
                        F[su, "expT"][:kr, kb * 256:kb * 256 + 256],
                        scp[:kr, kb * 256:kb * 256 + 256], ActF.Exp,
                        bias=vcol[:kr, su * 2 + kb:su * 2 + kb + 1],
                        scale=1.0)

            def av_block(su, dma_eng):
                K = slot_ks[su]
                KB = KBs[su]
                expT = F[su, "expT"]
                ot = F[su, "ot"]
                for qb in range(2):
                    av = avp.tile([128, 512], F32, tag="av", name="av")
                    for kb in range(KB):
                        kr = min(128, K - kb * 128)
                        nc.tensor.matmul(
                            av[:, 0:DV + 1],
                            expT[:kr, kb * 256 + qb * 128:
                                 kb * 256 + qb * 128 + 128],
                            vx(su, kb)[:kr, :],
                            start=(kb == 0), stop=(kb == KB - 1))
                    rec = postp.tile([128, 1], F32, tag="rec", name="rec")
                    nc.vector.reciprocal(rec[:], av[:, DV:DV + 1])
                    if qb == 0:
                        nc.vector.tensor_scalar_mul(
                            ot[:, qb * DV:qb * DV + DV],
                            av[:, 0:DV], rec[:, 0:1])
                    else:
                        nc.scalar.activation(ot[:, qb * DV:qb * DV + DV],
                                             av[:, 0:DV], ActF.Copy,
                                             scale=rec[:, 0:1])
                dma_eng.dma_start(out_exts[su], ot[:])

            proj_block(0)
            sins_block(0)
            proj_block(1)
            sins_block(1)
            dve_block(0)
            scores_block(0)
            exp_block(0)
            dve_block(1)
            scores_block(1)
            av_block(0, nc.sync)
            exp_block(1)
            av_block(1, nc.scalar)
    nc.compile()
    return nc


_CACHE = {}


def _get_graph(slot_ks):
    key = tuple(slot_ks)
    if key not in _CACHE:
        _CACHE[key] = _build(slot_ks)
    return _CACHE[key]


def _build_in_maps(queries, keys, values, valid_lens, W_q, W_k, w_v, slots):
    K0, K1 = slots[0][0], slots[1][0]
    KBs = [(K + 127) // 128 for K in (K0, K1)]
    wqk = np.empty((128, 1024), np.float16)
    for d in range(2):
        for hb in range(2):
            wqk[:, d * 256 + hb * 128:d * 256 + hb * 128 + 128] = \
                W_q[d * 128:(d + 1) * 128,
                    hb * 128:(hb + 1) * 128].astype(np.float16)
            wqk[:, 512 + d * 256 + hb * 128:512 + d * 256 + hb * 128 + 128] \
                = W_k[d * 128:(d + 1) * 128,
                      hb * 128:(hb + 1) * 128].astype(np.float16)
    wvs = np.empty((128, 6), np.float32)
    for m, a in enumerate((A1, A2, A3)):
        for hb in range(2):
            wvs[:, 2 * m + hb] = a * w_v[hb * 128:(hb + 1) * 128]
    wkv = W_k.astype(np.float32) @ (C0 * w_v.astype(np.float32))

    AW = 1024 + 2 * LQ + 2 * K0
    BW = 2 * LQ + 2 * K1 + KBs[0] * (DV + 1) + KBs[1] * (DV + 1)
    in_maps = []
    for c in range(N_CORES):
        arA = np.zeros((128, AW), np.float16)
        arA[:, 0:1024] = wqk
        arB = np.zeros((128, BW), np.float16)
        vcol = np.zeros((128, 4), np.float32)
        for su, (K, grp) in enumerate(slots):
            b = grp[c]
            vl = int(valid_lens[b])
            qT = queries[b].T.astype(np.float16)          # [D, LQ]
            kT = keys[b, :K, :].T.astype(np.float16)      # [D, K]
            base = 1024 if su == 0 else 0
            dst = arA if su == 0 else arB
            for d in range(2):
                dst[:, base + d * LQ:base + (d + 1) * LQ] = \
                    qT[d * 128:(d + 1) * 128]
                dst[:, base + 2 * LQ + d * K:base + 2 * LQ + (d + 1) * K] = \
                    kT[d * 128:(d + 1) * 128]
            KB = KBs[su]
            vpad = np.zeros((KB * 128, DV + 1), np.float16)
            vpad[:vl, :DV] = values[b, :vl, :].astype(np.float16)
            vpad[:vl, DV] = 1.0
            off = 2 * LQ + 2 * K1 + (KBs[0] * (DV + 1) if su else 0)
            arB[:, off:off + KB * (DV + 1)] = np.ascontiguousarray(
                vpad.reshape(KB, 128, DV + 1).transpose(1, 0, 2)
                .reshape(128, KB * (DV + 1)))
            v = (keys[b, :K, :].astype(np.float32) @ wkv)  # [K]
            for kb in range(KB):
                kr = min(128, K - kb * 128)
                vcol[:kr, su * 2 + kb] = v[kb * 128:kb * 128 + kr]
        in_maps.append({"arA": arA, "arB": arB, "wvs": wvs, "vcol": vcol})
    return in_maps


def kernel(queries, keys, values, valid_lens, W_q, W_k, w_v):
    queries = np.asarray(queries, dtype=np.float32)
    keys = np.asarray(keys, dtype=np.float32)
    values = np.asarray(values, dtype=np.float32)
    valid_lens = np.asarray(valid_lens)
    W_q = np.asarray(W_q, dtype=np.float32)
    W_k = np.asarray(W_k, dtype=np.float32)
    w_v = np.asarray(w_v, dtype=np.float32)

    slots = _plan(valid_lens)
    nc = _get_graph([K for (K, _) in slots])
    in_maps = _build_in_maps(queries, keys, values, valid_lens,
                             W_q, W_k, w_v, slots)
    res = run_bass_kernel_spmd(nc, in_maps, list(range(N_CORES)))

    out = np.empty((B, LQ, DV), np.float32)
    for su, (K, grp) in enumerate(slots):
        for c, b in enumerate(grp):
            o = res.results[c][f"out{su}"]        # [128, 2*DV] fp16
            out[b] = (o.reshape(128, 2, DV).transpose(1, 0, 2)
                      .reshape(LQ, DV).astype(np.float32))
    return out


# revision 19
# speedup vs baseline: 1.1190x; 1.0764x over previous
"""Additive attention (nn_AdditiveAttention) on 8 Trainium2 NeuronCores.

Math (per batch b):
  qp = queries[b] @ W_q ; kp = keys[b] @ W_k        # (L, H)
  S[q,k] = sum_h w_v[h] * tanh(qp[q,h] + kp[k,h])
  out[b] = softmax_k(S, masked to k < valid_lens[b]) @ values[b]

Approximation: tanh(x) ~= c0*x + sum_{m=1..3} a_m sin(m w x).
  - The linear term's q-part cancels in softmax; its k-part
    v_k = c0 * (keys @ W_k @ w_v) is host-precomputed and folded into
    the exp bias (scores land PSUM-[k,q], so a per-partition bias).
  - sin(mw(q+k)) = sin_m(q)cos_m(k) + cos_m(q)sin_m(k): one PE matmul
    contraction per product, depth 2*3*H over separable factors.
  - seeds on ScalarE: s1 = Sin(w*p), hh = Sin(w/2*p); cos1 = 1-2*hh^2
    (exact half-angle -- keeps every Sin argument inside the HW window).
  - m=2,3 by product identities on DVE: 2cos2 = 2-4*s1^2,
    sin2/2 = s1*c1, cos3 = (2cos2-1)*c1, sin3 = (2cos2+1)*s1 --
    fused scalar_tensor_tensor / two-op tensor_scalar instructions.
  - w_v*a_m folded into k-halves in place (per-hb tensor_scalar with a
    [128,1] column); m>=2 folds ride on GpSimd to unload DVE.
  - scores: 12-matmul PSUM accumulation group per (slot, kb);
    exp on ScalarE with bias = v-column; masked [values|1] matmul,
    DVE reciprocal + scale, fp16 output, one DMA per slot.

SPMD: one NEFF on 8 cores, 2 slots (one batch per core per slot).
Host sorts batches by valid_len; slot K = max valid_len of the slot.
"""

import sys

if "/opt/trn_rl_repo" not in sys.path:
    sys.path.insert(0, "/opt/trn_rl_repo")

import numpy as np

import concourse.bacc as bacc
import concourse.mybir as mybir
import concourse.tile as tile
from concourse.bass_utils import run_bass_kernel_spmd

N_CORES = 8
B, LQ, LK = 16, 256, 256
D = 256
H = 256
DV = 256
F32 = mybir.dt.float32
F16 = mybir.dt.float16

OMEGA = 0.8726646259971648          # 2*pi/7.2
C0 = 0.2757509648799896
A1 = 0.47873538732528687
A2 = 0.10933910310268402
A3 = 0.033587608486413956

Alu = mybir.AluOpType
ActF = mybir.ActivationFunctionType


def _plan(valid_lens):
    pieces = sorted(range(B), key=lambda b: -int(valid_lens[b]))
    slots = []
    for s in range(B // N_CORES):
        grp = pieces[s * N_CORES:(s + 1) * N_CORES]
        K = max(int(valid_lens[b]) for b in grp)
        K = min(LK, (K + 3) // 4 * 4)
        slots.append((K, grp))
    return slots


def _build(slot_ks):
    K0, K1 = slot_ks
    KBs = [(K + 127) // 128 for K in slot_ks]
    nc = bacc.Bacc("TRN2", target_bir_lowering=False, debug=False,
                   num_devices=N_CORES)
    # arenaA: [wqk(1024) | qT_d0(256) | qT_d1(256) | kT_d0(K0) | kT_d1(K0)]
    AW = 1024 + 2 * LQ + 2 * K0
    arA_ext = nc.dram_tensor("arA", [128, AW], F16, kind="ExternalInput").ap()
    # arenaB: [qkT1(512+2K1) | vx0(KB0*257) | vx1(KB1*257)]
    BW = 2 * LQ + 2 * K1 + KBs[0] * (DV + 1) + KBs[1] * (DV + 1)
    arB_ext = nc.dram_tensor("arB", [128, BW], F16, kind="ExternalInput").ap()
    wvs_ext = nc.dram_tensor("wvs", [128, 6], F32, kind="ExternalInput").ap()
    vcol_ext = nc.dram_tensor("vcol", [128, 4], F32,
                              kind="ExternalInput").ap()
    out_exts = [nc.dram_tensor(f"out{su}", [128, 2 * DV], F16,
                               kind="ExternalOutput").ap()
                for su in range(2)]

    with tile.TileContext(nc) as tc:
        with (
            tc.tile_pool(name="consts", bufs=1) as consts,
            tc.tile_pool(name="sb", bufs=1) as sb,
            tc.tile_pool(name="post", bufs=2) as postp,
            tc.tile_pool(name="pjp", bufs=2, space="PSUM") as pjp,
            tc.tile_pool(name="scp", bufs=2, space="PSUM") as scpp,
            tc.tile_pool(name="avp", bufs=2, space="PSUM") as avp,
        ):
            arA = consts.tile([128, AW], F16, tag="arA", name="arA")
            arB = consts.tile([128, BW], F16, tag="arB", name="arB")
            wvs = consts.tile([128, 6], F32, tag="wvs", name="wvs")
            vcol = consts.tile([128, 4], F32, tag="vcol", name="vcol")

            # critical input (wqk|qT|kT) split as FIRST transfer on each of
            # the 3 DMA queues; slot1/values arena + small consts second
            nc.sync.dma_start(arA[:, 0:1024], arA_ext[:, 0:1024])
            nc.gpsimd.dma_start(arA[:, 1024:1024 + 2 * LQ],
                                arA_ext[:, 1024:1024 + 2 * LQ])
            nc.scalar.dma_start(arA[:, 1024 + 2 * LQ:],
                                arA_ext[:, 1024 + 2 * LQ:])
            nc.scalar.dma_start(arB[:], arB_ext)
            nc.gpsimd.dma_start(wvs[:], wvs_ext)
            nc.gpsimd.dma_start(vcol[:], vcol_ext)

            def wq(d, hb):
                return arA[:, d * 256 + hb * 128:d * 256 + hb * 128 + 128]

            def wk(d, hb):
                return arA[:, 512 + d * 256 + hb * 128:
                           512 + d * 256 + hb * 128 + 128]

            def qkT(su, part, d):
                # part 0 = q (LQ cols), part 1 = k (K cols)
                K = slot_ks[su]
                base = 1024 if su == 0 else 0
                src = arA if su == 0 else arB
                off = base + (d * LQ if part == 0 else 2 * LQ + d * K)
                n = LQ if part == 0 else K
                return src[:, off:off + n]

            vx_base = 2 * LQ + 2 * K1

            def vx(su, kb):
                off = vx_base + (KBs[0] * (DV + 1) if su else 0) \
                    + kb * (DV + 1)
                return arB[:, off:off + DV + 1]

            # fused trig tiles per slot: [q_hb0|q_hb1|k_hb0|k_hb1]
            KO = 2 * LQ
            F = {}
            for su, K in enumerate(slot_ks):
                W = 2 * LQ + 2 * K
                for nm in ("F1s", "Fh", "F1c", "C4", "F3c", "F3s"):
                    F[su, nm] = sb.tile([128, W], F16, tag=f"{nm}{su}",
                                        name=f"{nm}{su}")
                F[su, "S2q"] = sb.tile([128, 2 * LQ], F16, tag=f"S2q{su}",
                                       name=f"S2q{su}")
                F[su, "K2s"] = sb.tile([128, 2 * K], F16, tag=f"K2s{su}",
                                       name=f"K2s{su}")
                F[su, "K2c"] = sb.tile([128, 2 * K], F16, tag=f"K2c{su}",
                                       name=f"K2c{su}")
                F[su, "expT"] = sb.tile([128, 2 * LQ], F16, tag=f"e{su}",
                                        name=f"e{su}")
                F[su, "ot"] = sb.tile([128, 2 * DV], F16, tag=f"ot{su}",
                                      name=f"ot{su}")

            # HAM warmup: dependency-free matmuls fill the DMA-wait window
            # and flip the PE clock gate to 8/8 before real work arrives.
            warm_sb = consts.tile([128, 2 * LQ], F16, tag="warm", name="warm")
            nc.vector.memset(warm_sb[:], 0.125)
            warm_ps = scpp.tile([128, 512], F32, tag="sc", name="warmps")
            NWARM = 28
            for i in range(NWARM):
                nc.tensor.matmul(warm_ps[:, 0:256],
                                 warm_sb[:, (i % 2) * 128:(i % 2) * 128 + 128],
                                 warm_sb[:, 256:512],
                                 start=(i == 0), stop=(i == NWARM - 1),
                                 skip_group_check=True)

            pskq = {}

            def proj_block(su):
                K = slot_ks[su]
                p = pjp.tile([128, 1024], F32, tag="pj", name=f"pj{su}")
                pskq[su] = p
                for hb in range(2):
                    for d in range(2):
                        nc.tensor.matmul(p[:, hb * 256:hb * 256 + 256],
                                         wq(d, hb), qkT(su, 0, d),
                                         start=(hb == 0 and d == 0),
                                         stop=False, skip_group_check=True)
                for hb in range(2):
                    for d in range(2):
                        nc.tensor.matmul(p[:, 512 + hb * K:512 + hb * K + K],
                                         wk(d, hb), qkT(su, 1, d),
                                         start=(hb == 0 and d == 0),
                                         stop=(hb == 1 and d == 1),
                                         skip_group_check=True)

            def sins_block(su):
                K = slot_ks[su]
                W = 2 * LQ + 2 * K
                p = pskq[su]
                nc.scalar.activation(F[su, "Fh"][:], p[:, 0:W], ActF.Sin,
                                     bias=0.0, scale=OMEGA / 2.0)
                nc.scalar.activation(F[su, "F1s"][:], p[:, 0:W], ActF.Sin,
                                     bias=0.0, scale=OMEGA)

            def dve_block(su):
                K = slot_ks[su]
                f1s, fh, f1c = F[su, "F1s"], F[su, "Fh"], F[su, "F1c"]
                c4, f3c, f3s = F[su, "C4"], F[su, "F3c"], F[su, "F3s"]
                k2s, k2c = F[su, "K2s"], F[su, "K2c"]
                kk = slice(KO, KO + 2 * K)
                qq = slice(0, 2 * LQ)
                tt = nc.vector.tensor_tensor
                ts = nc.vector.tensor_scalar
                tsm = nc.vector.tensor_scalar_mul
                # U (into f1c) and V (into c4) read unfolded seeds
                tt(f1c[:], fh[:], fh[:], Alu.mult)
                ts(f1c[:], f1c[:], -2.0, 1.0, Alu.mult, Alu.add)   # cos1
                for hb in range(2):                      # K1c = a1wv*c1k
                    ks = slice(KO + hb * K, KO + hb * K + K)
                    tsm(f1c[:, ks], f1c[:, ks], wvs[:, hb:hb + 1])
                tt(c4[:], f1s[:], f1s[:], Alu.mult)      # V (s1 unfolded)
                # K2s = s1k*(a1wv*c1k) * (a2/a1)   (s1k still unfolded)
                tt(k2s[:], f1s[:, kk], f1c[:, kk], Alu.mult)
                for hb in range(2):                      # K1s = a1wv*s1k
                    ks = slice(KO + hb * K, KO + hb * K + K)
                    tsm(f1s[:, ks], f1s[:, ks], wvs[:, hb:hb + 1])
                tsm(k2s[:], k2s[:], A2 / A1)
                ts(c4[:], c4[:], -4.0, 2.0, Alu.mult, Alu.add)     # 2cos2
                for hb in range(2):                      # K2c = a2wv*C4k
                    ks = slice(KO + hb * K, KO + hb * K + K)
                    tsm(k2c[:, hb * K:hb * K + K], c4[:, ks],
                        wvs[:, 2 + hb:3 + hb])
                # k-halves of m=3 first (they gate the score stationaries)
                ts(f3c[:, kk], c4[:, kk], 1.0, -1.0, Alu.mult, Alu.add)
                tt(f3c[:, kk], f3c[:, kk], f1c[:, kk], Alu.mult)
                tsm(f3c[:, kk], f3c[:, kk], A3 / A1)     # K3c
                ts(f3s[:, kk], c4[:, kk], 1.0, 1.0, Alu.mult, Alu.add)
                tt(f3s[:, kk], f3s[:, kk], f1s[:, kk], Alu.mult)
                tsm(f3s[:, kk], f3s[:, kk], A3 / A1)     # K3s
                tt(F[su, "S2q"][:], f1s[:, qq], f1c[:, qq], Alu.mult)
                ts(f3c[:, qq], c4[:, qq], 1.0, -1.0, Alu.mult, Alu.add)
                tt(f3c[:, qq], f3c[:, qq], f1c[:, qq], Alu.mult)
                ts(f3s[:, qq], c4[:, qq], 1.0, 1.0, Alu.mult, Alu.add)
                tt(f3s[:, qq], f3s[:, qq], f1s[:, qq], Alu.mult)

            scps = {}

            def scores_block(su):
                K = slot_ks[su]
                KB = KBs[su]
                pairs = (("F1c", "F1s"), ("F1s", "F1c"),
                         ("K2c", "S2q"), ("K2s", "C4"),
                         ("F3c", "F3s"), ("F3s", "F3c"))
                for kb in range(KB):
                    kr = min(128, K - kb * 128)
                    scp = scpp.tile([128, 512], F32, tag="sc",
                                    name=f"sc{su}_{kb}")
                    scps[su, kb] = scp
                    n = 0
                    for stat_nm, mov_nm in pairs:
                        for hb in range(2):
                            st = F[su, stat_nm]
                            if stat_nm in ("K2s", "K2c"):
                                stat = st[:, hb * K + kb * 128:
                                          hb * K + kb * 128 + kr]
                            else:
                                stat = st[:, KO + hb * K + kb * 128:
                                          KO + hb * K + kb * 128 + kr]
                            mov = F[su, mov_nm][:, hb * 256:hb * 256 + 256]
                            nc.tensor.matmul(
                                scp[:kr, 0:256],
                                stat, mov, start=(n == 0), stop=(n == 11),
                                skip_group_check=True)
                            n += 1

            def exp_block(su):
                K = slot_ks[su]
                KB = KBs[su]
                for kb in range(KB):
                    kr = min(128, K - kb * 128)
                    nc.scalar.activation(
                        F[su, "expT"][:kr, kb * 256:kb * 256 + 256],
                        scps[su, kb][:kr, 0:256], ActF.Exp,
                        bias=vcol[:kr, su * 2 + kb:su * 2 + kb + 1],
                        scale=1.0)

            def av_block(su, dma_eng):
                K = slot_ks[su]
                KB = KBs[su]
                expT = F[su, "expT"]
                ot = F[su, "ot"]
                for qb in range(2):
                    av = avp.tile([128, 512], F32, tag="av", name="av")
                    for kb in range(KB):
                        kr = min(128, K - kb * 128)
                        nc.tensor.matmul(
                            av[:, 0:DV + 1],
                            expT[:kr, kb * 256 + qb * 128:
                                 kb * 256 + qb * 128 + 128],
                            vx(su, kb)[:kr, :],
                            start=(kb == 0), stop=(kb == KB - 1))
                    rec = postp.tile([128, 1], F32, tag="rec", name="rec")
                    nc.vector.reciprocal(rec[:], av[:, DV:DV + 1])
                    if qb == 0:
                        nc.vector.tensor_scalar_mul(
                            ot[:, qb * DV:qb * DV + DV],
                            av[:, 0:DV], rec[:, 0:1])
                    else:
                        nc.scalar.activation(ot[:, qb * DV:qb * DV + DV],
                                             av[:, 0:DV], ActF.Copy,
                                             scale=rec[:, 0:1])
                dma_eng.dma_start(out_exts[su], ot[:])

            proj_block(0)
            sins_block(0)
            proj_block(1)
            sins_block(1)
            dve_block(0)
            scores_block(0)
            exp_block(0)
            dve_block(1)
            scores_block(1)
            av_block(0, nc.sync)
            exp_block(1)
            av_block(1, nc.scalar)
    nc.compile()
    return nc


_CACHE = {}


def _get_graph(slot_ks):
    key = tuple(slot_ks)
    if key not in _CACHE:
        _CACHE[key] = _build(slot_ks)
    return _CACHE[key]


def _build_in_maps(queries, keys, values, valid_lens, W_q, W_k, w_v, slots):
    K0, K1 = slots[0][0], slots[1][0]
    KBs = [(K + 127) // 128 for K in (K0, K1)]
    wqk = np.empty((128, 1024), np.float16)
    for d in range(2):
        for hb in range(2):
            wqk[:, d * 256 + hb * 128:d * 256 + hb * 128 + 128] = \
                W_q[d * 128:(d + 1) * 128,
                    hb * 128:(hb + 1) * 128].astype(np.float16)
            wqk[:, 512 + d * 256 + hb * 128:512 + d * 256 + hb * 128 + 128] \
                = W_k[d * 128:(d + 1) * 128,
                      hb * 128:(hb + 1) * 128].astype(np.float16)
    wvs = np.empty((128, 6), np.float32)
    for m, a in enumerate((A1, A2, A3)):
        for hb in range(2):
            wvs[:, 2 * m + hb] = a * w_v[hb * 128:(hb + 1) * 128]
    wkv = W_k.astype(np.float32) @ (C0 * w_v.astype(np.float32))

    AW = 1024 + 2 * LQ + 2 * K0
    BW = 2 * LQ + 2 * K1 + KBs[0] * (DV + 1) + KBs[1] * (DV + 1)
    in_maps = []
    for c in range(N_CORES):
        arA = np.zeros((128, AW), np.float16)
        arA[:, 0:1024] = wqk
        arB = np.zeros((128, BW), np.float16)
        vcol = np.zeros((128, 4), np.float32)
        for su, (K, grp) in enumerate(slots):
            b = grp[c]
            vl = int(valid_lens[b])
            qT = queries[b].T.astype(np.float16)          # [D, LQ]
            kT = keys[b, :K, :].T.astype(np.float16)      # [D, K]
            base = 1024 if su == 0 else 0
            dst = arA if su == 0 else arB
            for d in range(2):
                dst[:, base + d * LQ:base + (d + 1) * LQ] = \
                    qT[d * 128:(d + 1) * 128]
                dst[:, base + 2 * LQ + d * K:base + 2 * LQ + (d + 1) * K] = \
                    kT[d * 128:(d + 1) * 128]
            KB = KBs[su]
            vpad = np.zeros((KB * 128, DV + 1), np.float16)
            vpad[:vl, :DV] = values[b, :vl, :].astype(np.float16)
            vpad[:vl, DV] = 1.0
            off = 2 * LQ + 2 * K1 + (KBs[0] * (DV + 1) if su else 0)
            arB[:, off:off + KB * (DV + 1)] = np.ascontiguousarray(
                vpad.reshape(KB, 128, DV + 1).transpose(1, 0, 2)
                .reshape(128, KB * (DV + 1)))
            v = (keys[b, :K, :].astype(np.float32) @ wkv)  # [K]
            for kb in range(KB):
                kr = min(128, K - kb * 128)
                vcol[:kr, su * 2 + kb] = v[kb * 128:kb * 128 + kr]
        in_maps.append({"arA": arA, "arB": arB, "wvs": wvs, "vcol": vcol})
    return in_maps


def kernel(queries, keys, values, valid_lens, W_q, W_k, w_v):
    queries = np.asarray(queries, dtype=np.float32)
    keys = np.asarray(keys, dtype=np.float32)
    values = np.asarray(values, dtype=np.float32)
    valid_lens = np.asarray(valid_lens)
    W_q = np.asarray(W_q, dtype=np.float32)
    W_k = np.asarray(W_k, dtype=np.float32)
    w_v = np.asarray(w_v, dtype=np.float32)

    slots = _plan(valid_lens)
    nc = _get_graph([K for (K, _) in slots])
    in_maps = _build_in_maps(queries, keys, values, valid_lens,
                             W_q, W_k, w_v, slots)
    res = run_bass_kernel_spmd(nc, in_maps, list(range(N_CORES)))

    out = np.empty((B, LQ, DV), np.float32)
    for su, (K, grp) in enumerate(slots):
        for c, b in enumerate(grp):
            o = res.results[c][f"out{su}"]        # [128, 2*DV] fp16
            out[b] = (o.reshape(128, 2, DV).transpose(1, 0, 2)
                      .reshape(LQ, DV).astype(np.float32))
    return out


# revision 22
# speedup vs baseline: 1.1577x; 1.0346x over previous
"""Additive attention (nn_AdditiveAttention) on 8 Trainium2 NeuronCores.

Math (per batch b):
  qp = queries[b] @ W_q ; kp = keys[b] @ W_k        # (L, H)
  S[q,k] = sum_h w_v[h] * tanh(qp[q,h] + kp[k,h])
  out[b] = softmax_k(S, masked to k < valid_lens[b]) @ values[b]

Approximation: tanh(x) ~= c0*x + sum_{m=1..3} a_m sin(m w x).
  - The linear term's q-part cancels in softmax; its k-part
    v_k = c0 * (keys @ W_k @ w_v) is host-precomputed and folded into
    the exp bias (scores land PSUM-[k,q], so a per-partition bias).
  - sin(mw(q+k)) = sin_m(q)cos_m(k) + cos_m(q)sin_m(k): one PE matmul
    contraction per product, depth 2*3*H over separable factors.
  - seeds on ScalarE: s1 = Sin(w*p), hh = Sin(w/2*p); cos1 = 1-2*hh^2
    (exact half-angle -- keeps every Sin argument inside the HW window).
  - m=2,3 by product identities on DVE: 2cos2 = 2-4*s1^2,
    sin2/2 = s1*c1, cos3 = (2cos2-1)*c1, sin3 = (2cos2+1)*s1 --
    fused scalar_tensor_tensor / two-op tensor_scalar instructions.
  - w_v*a_m folded into k-halves in place (per-hb tensor_scalar with a
    [128,1] column); m>=2 folds ride on GpSimd to unload DVE.
  - scores: 12-matmul PSUM accumulation group per (slot, kb);
    exp on ScalarE with bias = v-column; masked [values|1] matmul,
    DVE reciprocal + scale, fp16 output, one DMA per slot.

SPMD: one NEFF on 8 cores, 2 slots (one batch per core per slot).
Host sorts batches by valid_len; slot K = max valid_len of the slot.
"""

import sys

if "/opt/trn_rl_repo" not in sys.path:
    sys.path.insert(0, "/opt/trn_rl_repo")

import numpy as np

import concourse.bacc as bacc
import concourse.mybir as mybir
import concourse.tile as tile
from concourse.bass_utils import run_bass_kernel_spmd

N_CORES = 8
B, LQ, LK = 16, 256, 256
D = 256
H = 256
DV = 256
F32 = mybir.dt.float32
F16 = mybir.dt.float16

OMEGA = 0.8726646259971648          # 2*pi/7.2
C0 = 0.2757509648799896
A1 = 0.47873538732528687
A2 = 0.10933910310268402
A3 = 0.033587608486413956

Alu = mybir.AluOpType
ActF = mybir.ActivationFunctionType


def _plan(valid_lens):
    pieces = sorted(range(B), key=lambda b: -int(valid_lens[b]))
    slots = []
    for s in range(B // N_CORES):
        grp = pieces[s * N_CORES:(s + 1) * N_CORES]
        K = max(int(valid_lens[b]) for b in grp)
        K = min(LK, (K + 3) // 4 * 4)
        slots.append((K, grp))
    return slots


def _build(slot_ks):
    K0, K1 = slot_ks
    KBs = [(K + 127) // 128 for K in slot_ks]
    nc = bacc.Bacc("TRN2", target_bir_lowering=False, debug=False,
                   num_devices=N_CORES)
    # arenaA: [wqk(1024) | qT_d0(256) | qT_d1(256) | kT_d0(K0) | kT_d1(K0)]
    AW = 1024 + 2 * LQ + 2 * K0
    arA_ext = nc.dram_tensor("arA", [128, AW], F16, kind="ExternalInput").ap()
    # arenaB: [qkT1(512+2K1) | vx0(KB0*257) | vx1(KB1*257)]
    BW = 2 * LQ + 2 * K1 + KBs[0] * (DV + 1) + KBs[1] * (DV + 1)
    arB_ext = nc.dram_tensor("arB", [128, BW], F16, kind="ExternalInput").ap()
    wvs_ext = nc.dram_tensor("wvs", [128, 6], F32, kind="ExternalInput").ap()
    vcol_ext = nc.dram_tensor("vcol", [128, 4], F32,
                              kind="ExternalInput").ap()
    out_exts = [nc.dram_tensor(f"out{su}", [128, 2 * DV], F16,
                               kind="ExternalOutput").ap()
                for su in range(2)]

    with tile.TileContext(nc) as tc:
        with (
            tc.tile_pool(name="consts", bufs=1) as consts,
            tc.tile_pool(name="sb", bufs=1) as sb,
            tc.tile_pool(name="post", bufs=2) as postp,
            tc.tile_pool(name="pjp", bufs=2, space="PSUM") as pjp,
            tc.tile_pool(name="scp", bufs=2, space="PSUM") as scpp,
            tc.tile_pool(name="avp", bufs=2, space="PSUM") as avp,
        ):
            arA = consts.tile([128, AW], F16, tag="arA", name="arA")
            arB = consts.tile([128, BW], F16, tag="arB", name="arB")
            wvs = consts.tile([128, 6], F32, tag="wvs", name="wvs")
            vcol = consts.tile([128, 4], F32, tag="vcol", name="vcol")

            # critical input (wqk|qT|kT) split as FIRST transfer on each of
            # the 3 DMA queues; slot1/values arena + small consts second
            nc.sync.dma_start(arA[:, 0:1024], arA_ext[:, 0:1024])
            nc.gpsimd.dma_start(arA[:, 1024:1024 + 2 * LQ],
                                arA_ext[:, 1024:1024 + 2 * LQ])
            nc.scalar.dma_start(arA[:, 1024 + 2 * LQ:],
                                arA_ext[:, 1024 + 2 * LQ:])
            nc.scalar.dma_start(arB[:], arB_ext)
            nc.gpsimd.dma_start(wvs[:], wvs_ext)
            nc.gpsimd.dma_start(vcol[:], vcol_ext)

            def wq(d, hb):
                return arA[:, d * 256 + hb * 128:d * 256 + hb * 128 + 128]

            def wk(d, hb):
                return arA[:, 512 + d * 256 + hb * 128:
                           512 + d * 256 + hb * 128 + 128]

            def qkT(su, part, d):
                # part 0 = q (LQ cols), part 1 = k (K cols)
                K = slot_ks[su]
                base = 1024 if su == 0 else 0
                src = arA if su == 0 else arB
                off = base + (d * LQ if part == 0 else 2 * LQ + d * K)
                n = LQ if part == 0 else K
                return src[:, off:off + n]

            vx_base = 2 * LQ + 2 * K1

            def vx(su, kb):
                off = vx_base + (KBs[0] * (DV + 1) if su else 0) \
                    + kb * (DV + 1)
                return arB[:, off:off + DV + 1]

            # fused trig tiles per slot: [q_hb0|q_hb1|k_hb0|k_hb1]
            KO = 2 * LQ
            F = {}
            for su, K in enumerate(slot_ks):
                W = 2 * LQ + 2 * K
                for nm in ("F1s", "Fh", "F1c", "C4", "F3c", "F3s"):
                    F[su, nm] = sb.tile([128, W], F16, tag=f"{nm}{su}",
                                        name=f"{nm}{su}")
                F[su, "S2q"] = sb.tile([128, 2 * LQ], F16, tag=f"S2q{su}",
                                       name=f"S2q{su}")
                F[su, "K2s"] = sb.tile([128, 2 * K], F16, tag=f"K2s{su}",
                                       name=f"K2s{su}")
                F[su, "K2c"] = sb.tile([128, 2 * K], F16, tag=f"K2c{su}",
                                       name=f"K2c{su}")
                F[su, "K2r"] = sb.tile([128, 2 * K], F16, tag=f"K2r{su}",
                                       name=f"K2r{su}")
                F[su, "expT"] = sb.tile([128, 2 * LQ], F16, tag=f"e{su}",
                                        name=f"e{su}")
                F[su, "ot"] = sb.tile([128, 2 * DV], F16, tag=f"ot{su}",
                                      name=f"ot{su}")

            # HAM warmup: dependency-free matmuls fill the DMA-wait window
            # and flip the PE clock gate to 8/8 before real work arrives.
            warm_sb = consts.tile([128, 2 * LQ], F16, tag="warm", name="warm")
            nc.vector.memset(warm_sb[:], 0.125)
            warm_ps = scpp.tile([128, 512], F32, tag="sc", name="warmps")
            NWARM = 22
            for i in range(NWARM):
                nc.tensor.matmul(warm_ps[:, 0:256],
                                 warm_sb[:, (i % 2) * 128:(i % 2) * 128 + 128],
                                 warm_sb[:, 256:512],
                                 start=(i == 0), stop=(i == NWARM - 1),
                                 skip_group_check=True)

            pskq = {}

            def proj_block(su):
                K = slot_ks[su]
                p = pjp.tile([128, 1024], F32, tag="pj", name=f"pj{su}")
                pskq[su] = p
                for hb in range(2):
                    for d in range(2):
                        nc.tensor.matmul(p[:, hb * 256:hb * 256 + 256],
                                         wq(d, hb), qkT(su, 0, d),
                                         start=(hb == 0 and d == 0),
                                         stop=False, skip_group_check=True)
                for hb in range(2):
                    for d in range(2):
                        nc.tensor.matmul(p[:, 512 + hb * K:512 + hb * K + K],
                                         wk(d, hb), qkT(su, 1, d),
                                         start=(hb == 0 and d == 0),
                                         stop=(hb == 1 and d == 1),
                                         skip_group_check=True)

            def sins_block(su):
                K = slot_ks[su]
                W = 2 * LQ + 2 * K
                p = pskq[su]
                nc.scalar.activation(F[su, "Fh"][:], p[:, 0:W], ActF.Sin,
                                     bias=0.0, scale=OMEGA / 2.0)
                nc.scalar.activation(F[su, "F1s"][:], p[:, 0:W], ActF.Sin,
                                     bias=0.0, scale=OMEGA)

            def dve_block(su):
                K = slot_ks[su]
                f1s, fh, f1c = F[su, "F1s"], F[su, "Fh"], F[su, "F1c"]
                c4, f3c, f3s = F[su, "C4"], F[su, "F3c"], F[su, "F3s"]
                k2s, k2c = F[su, "K2s"], F[su, "K2c"]
                kk = slice(KO, KO + 2 * K)
                qq = slice(0, 2 * LQ)
                tt = nc.vector.tensor_tensor
                ts = nc.vector.tensor_scalar
                tsm = nc.vector.tensor_scalar_mul
                # U (into f1c) and V (into c4) read unfolded seeds
                tt(f1c[:], fh[:], fh[:], Alu.mult)
                ts(f1c[:], f1c[:], -2.0, 1.0, Alu.mult, Alu.add)   # cos1
                for hb in range(2):                      # K1c = a1wv*c1k
                    ks = slice(KO + hb * K, KO + hb * K + K)
                    tsm(f1c[:, ks], f1c[:, ks], wvs[:, hb:hb + 1])
                tt(c4[:], f1s[:], f1s[:], Alu.mult)      # V (s1 unfolded)
                # K2s = s1k*(a1wv*c1k); ratio a2/a1 applied into K2r
                tt(k2s[:], f1s[:, kk], f1c[:, kk], Alu.mult)
                for hb in range(2):                      # K1s = a1wv*s1k
                    ks = slice(KO + hb * K, KO + hb * K + K)
                    tsm(f1s[:, ks], f1s[:, ks], wvs[:, hb:hb + 1])
                ts(c4[:], c4[:], -4.0, 2.0, Alu.mult, Alu.add)     # 2cos2
                if su == 0:
                    # slot0: ScalarE is idle after the sins -- offload the
                    # off-critical 1-input ops there
                    nc.scalar.activation(F[su, "K2r"][:], k2s[:], ActF.Copy,
                                         scale=A2 / A1)
                    for hb in range(2):
                        ks = slice(KO + hb * K, KO + hb * K + K)
                        nc.scalar.activation(k2c[:, hb * K:hb * K + K],
                                             c4[:, ks], ActF.Copy,
                                             scale=wvs[:, 2 + hb:3 + hb])
                else:
                    tsm(F[su, "K2r"][:], k2s[:], A2 / A1)
                    for hb in range(2):
                        ks = slice(KO + hb * K, KO + hb * K + K)
                        tsm(k2c[:, hb * K:hb * K + K], c4[:, ks],
                            wvs[:, 2 + hb:3 + hb])
                # k-halves of m=3 first (they gate the score stationaries)
                ts(f3c[:, kk], c4[:, kk], 1.0, -1.0, Alu.mult, Alu.add)
                tt(f3c[:, kk], f3c[:, kk], f1c[:, kk], Alu.mult)
                tsm(f3c[:, kk], f3c[:, kk], A3 / A1)     # K3c
                ts(f3s[:, kk], c4[:, kk], 1.0, 1.0, Alu.mult, Alu.add)
                tt(f3s[:, kk], f3s[:, kk], f1s[:, kk], Alu.mult)
                tsm(f3s[:, kk], f3s[:, kk], A3 / A1)     # K3s
                tt(F[su, "S2q"][:], f1s[:, qq], f1c[:, qq], Alu.mult)
                if su == 0:
                    nc.scalar.activation(f3c[:, qq], c4[:, qq], ActF.Copy,
                                         bias=-1.0, scale=1.0)
                    nc.scalar.activation(f3s[:, qq], c4[:, qq], ActF.Copy,
                                         bias=1.0, scale=1.0)
                else:
                    ts(f3c[:, qq], c4[:, qq], 1.0, -1.0, Alu.mult, Alu.add)
                    ts(f3s[:, qq], c4[:, qq], 1.0, 1.0, Alu.mult, Alu.add)
                tt(f3c[:, qq], f3c[:, qq], f1c[:, qq], Alu.mult)
                tt(f3s[:, qq], f3s[:, qq], f1s[:, qq], Alu.mult)

            scps = {}

            def scores_block(su):
                K = slot_ks[su]
                KB = KBs[su]
                pairs = (("F1c", "F1s"), ("F1s", "F1c"),
                         ("K2c", "S2q"), ("K2r", "C4"),
                         ("F3c", "F3s"), ("F3s", "F3c"))
                for kb in range(KB):
                    kr = min(128, K - kb * 128)
                    scp = scpp.tile([128, 512], F32, tag="sc",
                                    name=f"sc{su}_{kb}")
                    scps[su, kb] = scp
                    n = 0
                    for stat_nm, mov_nm in pairs:
                        for hb in range(2):
                            st = F[su, stat_nm]
                            if stat_nm in ("K2r", "K2c"):
                                stat = st[:, hb * K + kb * 128:
                                          hb * K + kb * 128 + kr]
                            else:
                                stat = st[:, KO + hb * K + kb * 128:
                                          KO + hb * K + kb * 128 + kr]
                            mov = F[su, mov_nm][:, hb * 256:hb * 256 + 256]
                            nc.tensor.matmul(
                                scp[:kr, 0:256],
                                stat, mov, start=(n == 0), stop=(n == 11),
                                skip_group_check=True)
                            n += 1

            def exp_block(su):
                K = slot_ks[su]
                KB = KBs[su]
                for kb in range(KB):
                    kr = min(128, K - kb * 128)
                    nc.scalar.activation(
                        F[su, "expT"][:kr, kb * 256:kb * 256 + 256],
                        scps[su, kb][:kr, 0:256], ActF.Exp,
                        bias=vcol[:kr, su * 2 + kb:su * 2 + kb + 1],
                        scale=1.0)

            def av_block(su, dma_eng):
                K = slot_ks[su]
                KB = KBs[su]
                expT = F[su, "expT"]
                ot = F[su, "ot"]
                for qb in range(2):
                    av = avp.tile([128, 512], F32, tag="av", name="av")
                    for kb in range(KB):
                        kr = min(128, K - kb * 128)
                        nc.tensor.matmul(
                            av[:, 0:DV + 1],
                            expT[:kr, kb * 256 + qb * 128:
                                 kb * 256 + qb * 128 + 128],
                            vx(su, kb)[:kr, :],
                            start=(kb == 0), stop=(kb == KB - 1))
                    rec = postp.tile([128, 1], F32, tag="rec", name="rec")
                    nc.vector.reciprocal(rec[:], av[:, DV:DV + 1])
                    if qb == 0:
                        nc.vector.tensor_scalar_mul(
                            ot[:, qb * DV:qb * DV + DV],
                            av[:, 0:DV], rec[:, 0:1])
                    else:
                        nc.scalar.activation(ot[:, qb * DV:qb * DV + DV],
                                             av[:, 0:DV], ActF.Copy,
                                             scale=rec[:, 0:1])
                dma_eng.dma_start(out_exts[su], ot[:])

            proj_block(0)
            sins_block(0)
            proj_block(1)
            sins_block(1)
            dve_block(0)
            scores_block(0)
            exp_block(0)
            dve_block(1)
            scores_block(1)
            av_block(0, nc.sync)
            exp_block(1)
            av_block(1, nc.scalar)
    nc.compile()
    return nc


_CACHE = {}


def _get_graph(slot_ks):
    key = tuple(slot_ks)
    if key not in _CACHE:
        _CACHE[key] = _build(slot_ks)
    return _CACHE[key]


def _build_in_maps(queries, keys, values, valid_lens, W_q, W_k, w_v, slots):
    K0, K1 = slots[0][0], slots[1][0]
    KBs = [(K + 127) // 128 for K in (K0, K1)]
    wqk = np.empty((128, 1024), np.float16)
    for d in range(2):
        for hb in range(2):
            wqk[:, d * 256 + hb * 128:d * 256 + hb * 128 + 128] = \
                W_q[d * 128:(d + 1) * 128,
                    hb * 128:(hb + 1) * 128].astype(np.float16)
            wqk[:, 512 + d * 256 + hb * 128:512 + d * 256 + hb * 128 + 128] \
                = W_k[d * 128:(d + 1) * 128,
                      hb * 128:(hb + 1) * 128].astype(np.float16)
    wvs = np.empty((128, 6), np.float32)
    for m, a in enumerate((A1, A2, A3)):
        for hb in range(2):
            wvs[:, 2 * m + hb] = a * w_v[hb * 128:(hb + 1) * 128]
    wkv = W_k.astype(np.float32) @ (C0 * w_v.astype(np.float32))

    AW = 1024 + 2 * LQ + 2 * K0
    BW = 2 * LQ + 2 * K1 + KBs[0] * (DV + 1) + KBs[1] * (DV + 1)
    in_maps = []
    for c in range(N_CORES):
        arA = np.zeros((128, AW), np.float16)
        arA[:, 0:1024] = wqk
        arB = np.zeros((128, BW), np.float16)
        vcol = np.zeros((128, 4), np.float32)
        for su, (K, grp) in enumerate(slots):
            b = grp[c]
            vl = int(valid_lens[b])
            qT = queries[b].T.astype(np.float16)          # [D, LQ]
            kT = keys[b, :K, :].T.astype(np.float16)      # [D, K]
            base = 1024 if su == 0 else 0
            dst = arA if su == 0 else arB
            for d in range(2):
                dst[:, base + d * LQ:base + (d + 1) * LQ] = \
                    qT[d * 128:(d + 1) * 128]
                dst[:, base + 2 * LQ + d * K:base + 2 * LQ + (d + 1) * K] = \
                    kT[d * 128:(d + 1) * 128]
            KB = KBs[su]
            vpad = np.zeros((KB * 128, DV + 1), np.float16)
            vpad[:vl, :DV] = values[b, :vl, :].astype(np.float16)
            vpad[:vl, DV] = 1.0
            off = 2 * LQ + 2 * K1 + (KBs[0] * (DV + 1) if su else 0)
            arB[:, off:off + KB * (DV + 1)] = np.ascontiguousarray(
                vpad.reshape(KB, 128, DV + 1).transpose(1, 0, 2)
                .reshape(128, KB * (DV + 1)))
            v = (keys[b, :K, :].astype(np.float32) @ wkv)  # [K]
            for kb in range(KB):
                kr = min(128, K - kb * 128)
                vcol[:kr, su * 2 + kb] = v[kb * 128:kb * 128 + kr]
        in_maps.append({"arA": arA, "arB": arB, "wvs": wvs, "vcol": vcol})
    return in_maps


def kernel(queries, keys, values, valid_lens, W_q, W_k, w_v):
    queries = np.asarray(queries, dtype=np.float32)
    keys = np.asarray(keys, dtype=np.float32)
    values = np.asarray(values, dtype=np.float32)
    valid_lens = np.asarray(valid_lens)
    W_q = np.asarray(W_q, dtype=np.float32)
    W_k = np.asarray(W_k, dtype=np.float32)
    w_v = np.asarray(w_v, dtype=np.float32)

    slots = _plan(valid_lens)
    nc = _get_graph([K for (K, _) in slots])
    in_maps = _build_in_maps(queries, keys, values, valid_lens,
                             W_q, W_k, w_v, slots)
    res = run_bass_kernel_spmd(nc, in_maps, list(range(N_CORES)))

    out = np.empty((B, LQ, DV), np.float32)
    for su, (K, grp) in enumerate(slots):
        for c, b in enumerate(grp):
            o = res.results[c][f"out{su}"]        # [128, 2*DV] fp16
            out[b] = (o.reshape(128, 2, DV).transpose(1, 0, 2)
                      .reshape(LQ, DV).astype(np.float32))
    return out


# revision 26
# speedup vs baseline: 1.1747x; 1.0147x over previous
"""Additive attention (nn_AdditiveAttention) on 8 Trainium2 NeuronCores.

Math (per batch b):
  qp = queries[b] @ W_q ; kp = keys[b] @ W_k        # (L, H)
  S[q,k] = sum_h w_v[h] * tanh(qp[q,h] + kp[k,h])
  out[b] = softmax_k(S, masked to k < valid_lens[b]) @ values[b]

Approximation: tanh(x) ~= c0*x + sum_{m=1..3} a_m sin(m w x).
  - The linear term's q-part cancels in softmax; its k-part
    v_k = c0 * (keys @ W_k @ w_v) is host-precomputed and folded into
    the exp bias (scores land PSUM-[k,q], so a per-partition bias).
  - sin(mw(q+k)) = sin_m(q)cos_m(k) + cos_m(q)sin_m(k): one PE matmul
    contraction per product, depth 2*3*H over separable factors.
  - seeds on ScalarE: s1 = Sin(w*p), hh = Sin(w/2*p); cos1 = 1-2*hh^2
    (exact half-angle -- keeps every Sin argument inside the HW window).
  - m=2,3 by product identities on DVE: 2cos2 = 2-4*s1^2,
    sin2/2 = s1*c1, cos3 = (2cos2-1)*c1, sin3 = (2cos2+1)*s1 --
    fused scalar_tensor_tensor / two-op tensor_scalar instructions.
  - w_v*a_m folded into k-halves in place (per-hb tensor_scalar with a
    [128,1] column); m>=2 folds ride on GpSimd to unload DVE.
  - scores: 12-matmul PSUM accumulation group per (slot, kb);
    exp on ScalarE with bias = v-column; masked [values|1] matmul,
    DVE reciprocal + scale, fp16 output, one DMA per slot.

SPMD: one NEFF on 8 cores, 2 slots (one batch per core per slot).
Host sorts batches by valid_len; slot K = max valid_len of the slot.
"""

import sys

if "/opt/trn_rl_repo" not in sys.path:
    sys.path.insert(0, "/opt/trn_rl_repo")

import numpy as np

import concourse.bacc as bacc
import concourse.mybir as mybir
import concourse.tile as tile
from concourse.bass_utils import run_bass_kernel_spmd

N_CORES = 8
B, LQ, LK = 16, 256, 256
D = 256
H = 256
DV = 256
F32 = mybir.dt.float32
F16 = mybir.dt.float16

OMEGA = 0.8726646259971648          # 2*pi/7.2
C0 = 0.2757509648799896
A1 = 0.47873538732528687
A2 = 0.10933910310268402
A3 = 0.033587608486413956

Alu = mybir.AluOpType
ActF = mybir.ActivationFunctionType


def _plan(valid_lens):
    pieces = sorted(range(B), key=lambda b: -int(valid_lens[b]))
    slots = []
    for s in range(B // N_CORES):
        grp = pieces[s * N_CORES:(s + 1) * N_CORES]
        K = max(int(valid_lens[b]) for b in grp)
        K = min(LK, (K + 3) // 4 * 4)
        slots.append((K, grp))
    return slots


def _build(slot_ks):
    K0, K1 = slot_ks
    KBs = [(K + 127) // 128 for K in slot_ks]
    nc = bacc.Bacc("TRN2", target_bir_lowering=False, debug=False,
                   num_devices=N_CORES)
    # arenaA: [wqk(1024) | qT_d0(256) | qT_d1(256) | kT_d0(K0) | kT_d1(K0)]
    AW = 1024 + 2 * LQ + 2 * K0
    arA_ext = nc.dram_tensor("arA", [128, AW], F16, kind="ExternalInput").ap()
    # arenaB: [qkT1(512+2K1) | vx0(KB0*257) | vx1(KB1*257)]
    BW = 2 * LQ + 2 * K1 + KBs[0] * (DV + 1) + KBs[1] * (DV + 1)
    arB_ext = nc.dram_tensor("arB", [128, BW], F16, kind="ExternalInput").ap()
    wvs_ext = nc.dram_tensor("wvs", [128, 6], F32, kind="ExternalInput").ap()
    vcol_ext = nc.dram_tensor("vcol", [128, 4], F32,
                              kind="ExternalInput").ap()
    out_exts = [nc.dram_tensor(f"out{su}", [128, 2 * DV], F16,
                               kind="ExternalOutput").ap()
                for su in range(2)]

    with tile.TileContext(nc) as tc:
        with (
            tc.tile_pool(name="consts", bufs=1) as consts,
            tc.tile_pool(name="sb", bufs=1) as sb,
            tc.tile_pool(name="post", bufs=2) as postp,
            tc.tile_pool(name="pjp", bufs=2, space="PSUM") as pjp,
            tc.tile_pool(name="scp", bufs=2, space="PSUM") as scpp,
            tc.tile_pool(name="avp", bufs=2, space="PSUM") as avp,
        ):
            arA = consts.tile([128, AW], F16, tag="arA", name="arA")
            arB = consts.tile([128, BW], F16, tag="arB", name="arB")
            wvs = consts.tile([128, 6], F32, tag="wvs", name="wvs")
            vcol = consts.tile([128, 4], F32, tag="vcol", name="vcol")

            # critical input (wqk|qT|kT) split as FIRST transfer on each of
            # the 3 DMA queues; slot1/values arena + small consts second
            nc.sync.dma_start(arA[:, 0:1024], arA_ext[:, 0:1024])
            nc.gpsimd.dma_start(arA[:, 1024:1024 + 2 * LQ],
                                arA_ext[:, 1024:1024 + 2 * LQ])
            nc.scalar.dma_start(arA[:, 1024 + 2 * LQ:],
                                arA_ext[:, 1024 + 2 * LQ:])
            nc.scalar.dma_start(arB[:], arB_ext)
            nc.gpsimd.dma_start(wvs[:], wvs_ext)
            nc.gpsimd.dma_start(vcol[:], vcol_ext)

            def wq(d, hb):
                return arA[:, d * 256 + hb * 128:d * 256 + hb * 128 + 128]

            def wk(d, hb):
                return arA[:, 512 + d * 256 + hb * 128:
                           512 + d * 256 + hb * 128 + 128]

            def qkT(su, part, d):
                # part 0 = q (LQ cols), part 1 = k (K cols)
                K = slot_ks[su]
                base = 1024 if su == 0 else 0
                src = arA if su == 0 else arB
                off = base + (d * LQ if part == 0 else 2 * LQ + d * K)
                n = LQ if part == 0 else K
                return src[:, off:off + n]

            vx_base = 2 * LQ + 2 * K1

            def vx(su, kb):
                off = vx_base + (KBs[0] * (DV + 1) if su else 0) \
                    + kb * (DV + 1)
                return arB[:, off:off + DV + 1]

            # fused trig tiles per slot: [q_hb0|q_hb1|k_hb0|k_hb1]
            KO = 2 * LQ
            F = {}
            for su, K in enumerate(slot_ks):
                W = 2 * LQ + 2 * K
                for nm in ("F1s", "Fh", "F1c", "C4", "F3c", "F3s"):
                    F[su, nm] = sb.tile([128, W], F16, tag=f"{nm}{su}",
                                        name=f"{nm}{su}")
                F[su, "S2q"] = sb.tile([128, 2 * LQ], F16, tag=f"S2q{su}",
                                       name=f"S2q{su}")
                F[su, "K2s"] = sb.tile([128, 2 * K], F16, tag=f"K2s{su}",
                                       name=f"K2s{su}")
                F[su, "K2c"] = sb.tile([128, 2 * K], F16, tag=f"K2c{su}",
                                       name=f"K2c{su}")
                F[su, "K2r"] = sb.tile([128, 2 * K], F16, tag=f"K2r{su}",
                                       name=f"K2r{su}")
                F[su, "expT"] = sb.tile([128, 2 * LQ], F16, tag=f"e{su}",
                                        name=f"e{su}")
                F[su, "ot"] = sb.tile([128, 2 * DV], F16, tag=f"ot{su}",
                                      name=f"ot{su}")

            # HAM warmup: dependency-free matmuls fill the DMA-wait window
            # and flip the PE clock gate to 8/8 before real work arrives.
            warm_sb = consts.tile([128, 2 * LQ], F16, tag="warm", name="warm")
            nc.vector.memset(warm_sb[:], 0.125)
            warm_ps = scpp.tile([128, 512], F32, tag="sc", name="warmps")
            NWARM = 22
            for i in range(NWARM):
                nc.tensor.matmul(warm_ps[:, 0:256],
                                 warm_sb[:, (i % 2) * 128:(i % 2) * 128 + 128],
                                 warm_sb[:, 256:512],
                                 start=(i == 0), stop=(i == NWARM - 1),
                                 skip_group_check=True)

            pskq = {}

            def proj_block(su):
                K = slot_ks[su]
                p = pjp.tile([128, 1024], F32, tag="pj", name=f"pj{su}")
                pskq[su] = p
                for hb in range(2):
                    for d in range(2):
                        nc.tensor.matmul(p[:, hb * 256:hb * 256 + 256],
                                         wq(d, hb), qkT(su, 0, d),
                                         start=(hb == 0 and d == 0),
                                         stop=False, skip_group_check=True)
                for hb in range(2):
                    for d in range(2):
                        nc.tensor.matmul(p[:, 512 + hb * K:512 + hb * K + K],
                                         wk(d, hb), qkT(su, 1, d),
                                         start=(hb == 0 and d == 0),
                                         stop=(hb == 1 and d == 1),
                                         skip_group_check=True)

            def sins_block(su):
                K = slot_ks[su]
                W = 2 * LQ + 2 * K
                p = pskq[su]
                import contextlib
                ctx = tc.high_priority() if su == 0 else contextlib.nullcontext()
                with ctx:
                    nc.scalar.activation(F[su, "Fh"][:], p[:, 0:W], ActF.Sin,
                                         bias=0.0, scale=OMEGA / 2.0)
                    nc.scalar.activation(F[su, "F1s"][:], p[:, 0:W], ActF.Sin,
                                         bias=0.0, scale=OMEGA)

            def dve_block(su):
                K = slot_ks[su]
                f1s, fh, f1c = F[su, "F1s"], F[su, "Fh"], F[su, "F1c"]
                c4, f3c, f3s = F[su, "C4"], F[su, "F3c"], F[su, "F3s"]
                k2s, k2c = F[su, "K2s"], F[su, "K2c"]
                kk = slice(KO, KO + 2 * K)
                qq = slice(0, 2 * LQ)
                tt = nc.vector.tensor_tensor
                ts = nc.vector.tensor_scalar
                tsm = nc.vector.tensor_scalar_mul
                # U (into f1c) and V (into c4) read unfolded seeds
                tt(f1c[:], fh[:], fh[:], Alu.mult)
                ts(f1c[:], f1c[:], -2.0, 1.0, Alu.mult, Alu.add)   # cos1
                for hb in range(2):                      # K1c = a1wv*c1k
                    ks = slice(KO + hb * K, KO + hb * K + K)
                    tsm(f1c[:, ks], f1c[:, ks], wvs[:, hb:hb + 1])
                tt(c4[:], f1s[:], f1s[:], Alu.mult)      # V (s1 unfolded)
                # K2s = s1k*(a1wv*c1k); ratio a2/a1 applied into K2r
                tt(k2s[:], f1s[:, kk], f1c[:, kk], Alu.mult)
                for hb in range(2):                      # K1s = a1wv*s1k
                    ks = slice(KO + hb * K, KO + hb * K + K)
                    tsm(f1s[:, ks], f1s[:, ks], wvs[:, hb:hb + 1])
                ts(c4[:], c4[:], -4.0, 2.0, Alu.mult, Alu.add)     # 2cos2
                # ScalarE is idle between the sins and the exps -- offload
                # the off-critical 1-input ops there (both slots; slot1's
                # land after exp0 in the ACT FIFO, still in time)
                nc.scalar.activation(F[su, "K2r"][:], k2s[:], ActF.Copy,
                                     scale=A2 / A1)
                for hb in range(2):
                    ks = slice(KO + hb * K, KO + hb * K + K)
                    nc.scalar.activation(k2c[:, hb * K:hb * K + K],
                                         c4[:, ks], ActF.Copy,
                                         scale=wvs[:, 2 + hb:3 + hb])
                # k-halves of m=3 first (they gate the score stationaries)
                ts(f3c[:, kk], c4[:, kk], 1.0, -1.0, Alu.mult, Alu.add)
                tt(f3c[:, kk], f3c[:, kk], f1c[:, kk], Alu.mult)
                tsm(f3c[:, kk], f3c[:, kk], A3 / A1)     # K3c
                ts(f3s[:, kk], c4[:, kk], 1.0, 1.0, Alu.mult, Alu.add)
                tt(f3s[:, kk], f3s[:, kk], f1s[:, kk], Alu.mult)
                tsm(f3s[:, kk], f3s[:, kk], A3 / A1)     # K3s
                tt(F[su, "S2q"][:], f1s[:, qq], f1c[:, qq], Alu.mult)
                nc.scalar.activation(f3c[:, qq], c4[:, qq], ActF.Copy,
                                     bias=-1.0, scale=1.0)
                nc.scalar.activation(f3s[:, qq], c4[:, qq], ActF.Copy,
                                     bias=1.0, scale=1.0)
                tt(f3c[:, qq], f3c[:, qq], f1c[:, qq], Alu.mult)
                tt(f3s[:, qq], f3s[:, qq], f1s[:, qq], Alu.mult)

            scps = {}

            def scores_block(su):
                K = slot_ks[su]
                KB = KBs[su]
                pairs = (("F1c", "F1s"), ("F1s", "F1c"),
                         ("K2c", "S2q"), ("K2r", "C4"),
                         ("F3c", "F3s"), ("F3s", "F3c"))
                for kb in range(KB):
                    kr = min(128, K - kb * 128)
                    scp = scpp.tile([128, 512], F32, tag="sc",
                                    name=f"sc{su}_{kb}")
                    scps[su, kb] = scp
                    n = 0
                    for stat_nm, mov_nm in pairs:
                        for hb in range(2):
                            st = F[su, stat_nm]
                            if stat_nm in ("K2r", "K2c"):
                                stat = st[:, hb * K + kb * 128:
                                          hb * K + kb * 128 + kr]
                            else:
                                stat = st[:, KO + hb * K + kb * 128:
                                          KO + hb * K + kb * 128 + kr]
                            mov = F[su, mov_nm][:, hb * 256:hb * 256 + 256]
                            nc.tensor.matmul(
                                scp[:kr, 0:256],
                                stat, mov, start=(n == 0), stop=(n == 11),
                                skip_group_check=True)
                            n += 1

            def exp_block(su):
                K = slot_ks[su]
                KB = KBs[su]
                for kb in range(KB):
                    kr = min(128, K - kb * 128)
                    nc.scalar.activation(
                        F[su, "expT"][:kr, kb * 256:kb * 256 + 256],
                        scps[su, kb][:kr, 0:256], ActF.Exp,
                        bias=vcol[:kr, su * 2 + kb:su * 2 + kb + 1],
                        scale=1.0)

            def av_block(su, dma_eng):
                K = slot_ks[su]
                KB = KBs[su]
                expT = F[su, "expT"]
                ot = F[su, "ot"]
                for qb in range(2):
                    av = avp.tile([128, 512], F32, tag="av", name="av")
                    for kb in range(KB):
                        kr = min(128, K - kb * 128)
                        nc.tensor.matmul(
                            av[:, 0:DV + 1],
                            expT[:kr, kb * 256 + qb * 128:
                                 kb * 256 + qb * 128 + 128],
                            vx(su, kb)[:kr, :],
                            start=(kb == 0), stop=(kb == KB - 1))
                    rec = postp.tile([128, 1], F32, tag="rec", name="rec")
                    nc.vector.reciprocal(rec[:], av[:, DV:DV + 1])
                    if qb == 0 or su == 1:
                        nc.vector.tensor_scalar_mul(
                            ot[:, qb * DV:qb * DV + DV],
                            av[:, 0:DV], rec[:, 0:1])
                    else:
                        nc.scalar.activation(ot[:, qb * DV:qb * DV + DV],
                                             av[:, 0:DV], ActF.Copy,
                                             scale=rec[:, 0:1])
                    if su == 1:
                        dma_eng.dma_start(
                            out_exts[su][:, qb * DV:qb * DV + DV],
                            ot[:, qb * DV:qb * DV + DV])
                if su == 0:
                    dma_eng.dma_start(out_exts[su], ot[:])

            proj_block(0)
            sins_block(0)
            proj_block(1)
            sins_block(1)
            dve_block(0)
            scores_block(0)
            exp_block(0)
            dve_block(1)
            scores_block(1)
            av_block(0, nc.sync)
            exp_block(1)
            av_block(1, nc.scalar)
    nc.compile()
    return nc


_CACHE = {}


def _get_graph(slot_ks):
    key = tuple(slot_ks)
    if key not in _CACHE:
        _CACHE[key] = _build(slot_ks)
    return _CACHE[key]


def _build_in_maps(queries, keys, values, valid_lens, W_q, W_k, w_v, slots):
    K0, K1 = slots[0][0], slots[1][0]
    KBs = [(K + 127) // 128 for K in (K0, K1)]
    wqk = np.empty((128, 1024), np.float16)
    for d in range(2):
        for hb in range(2):
            wqk[:, d * 256 + hb * 128:d * 256 + hb * 128 + 128] = \
                W_q[d * 128:(d + 1) * 128,
                    hb * 128:(hb + 1) * 128].astype(np.float16)
            wqk[:, 512 + d * 256 + hb * 128:512 + d * 256 + hb * 128 + 128] \
                = W_k[d * 128:(d + 1) * 128,
                      hb * 128:(hb + 1) * 128].astype(np.float16)
    wvs = np.empty((128, 6), np.float32)
    for m, a in enumerate((A1, A2, A3)):
        for hb in range(2):
            wvs[:, 2 * m + hb] = a * w_v[hb * 128:(hb + 1) * 128]
    wkv = W_k.astype(np.float32) @ (C0 * w_v.astype(np.float32))

    AW = 1024 + 2 * LQ + 2 * K0
    BW = 2 * LQ + 2 * K1 + KBs[0] * (DV + 1) + KBs[1] * (DV + 1)
    in_maps = []
    for c in range(N_CORES):
        arA = np.zeros((128, AW), np.float16)
        arA[:, 0:1024] = wqk
        arB = np.zeros((128, BW), np.float16)
        vcol = np.zeros((128, 4), np.float32)
        for su, (K, grp) in enumerate(slots):
            b = grp[c]
            vl = int(valid_lens[b])
            qT = queries[b].T.astype(np.float16)          # [D, LQ]
            kT = keys[b, :K, :].T.astype(np.float16)      # [D, K]
            base = 1024 if su == 0 else 0
            dst = arA if su == 0 else arB
            for d in range(2):
                dst[:, base + d * LQ:base + (d + 1) * LQ] = \
                    qT[d * 128:(d + 1) * 128]
                dst[:, base + 2 * LQ + d * K:base + 2 * LQ + (d + 1) * K] = \
                    kT[d * 128:(d + 1) * 128]
            KB = KBs[su]
            vpad = np.zeros((KB * 128, DV + 1), np.float16)
            vpad[:vl, :DV] = values[b, :vl, :].astype(np.float16)
            vpad[:vl, DV] = 1.0
            off = 2 * LQ + 2 * K1 + (KBs[0] * (DV + 1) if su else 0)
            arB[:, off:off + KB * (DV + 1)] = np.ascontiguousarray(
                vpad.reshape(KB, 128, DV + 1).transpose(1, 0, 2)
                .reshape(128, KB * (DV + 1)))
            v = (keys[b, :K, :].astype(np.float32) @ wkv)  # [K]
            for kb in range(KB):
                kr = min(128, K - kb * 128)
                vcol[:kr, su * 2 + kb] = v[kb * 128:kb * 128 + kr]
        in_maps.append({"arA": arA, "arB": arB, "wvs": wvs, "vcol": vcol})
    return in_maps


def kernel(queries, keys, values, valid_lens, W_q, W_k, w_v):
    queries = np.asarray(queries, dtype=np.float32)
    keys = np.asarray(keys, dtype=np.float32)
    values = np.asarray(values, dtype=np.float32)
    valid_lens = np.asarray(valid_lens)
    W_q = np.asarray(W_q, dtype=np.float32)
    W_k = np.asarray(W_k, dtype=np.float32)
    w_v = np.asarray(w_v, dtype=np.float32)

    slots = _plan(valid_lens)
    nc = _get_graph([K for (K, _) in slots])
    in_maps = _build_in_maps(queries, keys, values, valid_lens,
                             W_q, W_k, w_v, slots)
    res = run_bass_kernel_spmd(nc, in_maps, list(range(N_CORES)))

    out = np.empty((B, LQ, DV), np.float32)
    for su, (K, grp) in enumerate(slots):
        for c, b in enumerate(grp):
            o = res.results[c][f"out{su}"]        # [128, 2*DV] fp16
            out[b] = (o.reshape(128, 2, DV).transpose(1, 0, 2)
                      .reshape(LQ, DV).astype(np.float32))
    return out
